# revision 4
# baseline (speedup 1.0000x reference)
"""DH-SRNN forward on 8 Trainium2 cores — fused pre/scan pipeline.

Data-parallel over batch (B=256 -> 32 rows/core), weights replicated.
Measured ~5.8-6.9 ms vs the 8.8 ms two-phase baseline. Key structure:
  - Wh in bf16: the 256 scan matmuls/step run at ~31ns/MM (vs 46ns fp8; the
    stationary-load path is faster for bf16, and fp8 DoubleRow loses at n=32).
  - Phase-1 (xp = x @ Wx'^T) is fused INTO the scan loop as a
    producer-consumer pipeline over 2 SBUF chunk buffers: chunk c+2's GEMM
    fills PE idle slots while chunk c's scan runs, and xp never touches DRAM.
  - xp enters d via PSUM preload (identity matmul, start=True) so the DVE
    d-update is 2 passes (beta-mult, psum-add) instead of 3.
  - Spikes for a chunk are kept in a history tile; the readout GEMM runs
    once per chunk with n=512 (8+1 matmuls) instead of per step with n=32.
  - The readout membrane filter (mem_ro = aro*mem_ro + roT) moved off the
    scan entirely: raw roT history goes to DRAM; the end-pass applies the
    exponential filter with tensor_tensor_scan along time (stride-32 view,
    in place), then softmax + warmup-masked sum.
  - Pipeline: CH=8-step chunks, 3 SBUF chunk buffers, chunk c+2 produced
    (4 pre jf-groups interleaved after each scan step, explicitly in program
    order - the tile scheduler does NOT fill PE gaps on its own) while chunk
    c is scanned; 2 chunks of slack hide the PSUM->SBUF evacuation latency.
  - PSUM: 6 banks for the scan's hp tiles (rotation depth 3 steps; fewer
    banks stall the PE), 2 for the pre GEMM.
  - mem update split: mp = alpha*mem - spk computed off-chain; on-chain
    mem = mp + l_half. d *= beta also runs off-chain before hp lands.

Math (host-side fold, exact):
  beta = sigmoid(tau_n)[H,BR], alpha = sigmoid(tau_m)[H], aro = sigmoid(tau_m_ro)
  f' = br*H + h (branch-major), c[f'] = (1-alpha[h])*(1-beta[h,br]) folded
  into W_dense rows/bias; D := (1-alpha)*d
     D_t   = beta*D_{t-1} + (xp'_t + spk_{t-1} @ Wh'^T)
     mem_t = alpha*mem_{t-1} + sum_br D_t - spk_{t-1}
     spk_t = (mem_t > 1)
     roT_t = Wro' @ spk_t + bro'        (Wro' = (1-aro)*W_ro etc.)
  end:  mem_ro_t = aro*mem_ro_{t-1} + roT_t ;  out = sum_{t>10} softmax(mem_ro_t)
"""

import numpy as np
import ml_dtypes

import concourse.bass as bass
import concourse.bacc as bacc
import concourse.mybir as mybir
import concourse.tile as tile
from concourse.bass_utils import run_bass_kernel_spmd

F32 = mybir.dt.float32
BF16 = mybir.dt.bfloat16
FP8 = mybir.dt.float8e4

B, T_FULL, IN_DIM = 256, 500, 700
H, BR, O = 1024, 4, 20
NCORES = 8
BL = B // NCORES            # 32 batch rows per core
KT = 6                      # k-tiles for input dim (700 + bias row -> 768)
KIN = KT * 128              # 768
JF = (H * BR) // 128        # 32 feature blocks
JH = H // 128               # 8 hidden blocks
CH = 8                      # timesteps per chunk
NCH = CH * BL               # 512 xT cols per chunk
CHW = JF * NCH              # xp cols per chunk (16384)
SPW = JH * BL               # spike cols per step (256)
PAD_CHK = 2                 # extra xT chunks so the producer can overrun
WARMUP = 10
WX_SCALE = 1024.0           # lifts Wx into fp8e4m3 normal range; undone by
                            # ident = eye/WX_SCALE in the xp-preload matmul


def _sigmoid(x):
    return 1.0 / (1.0 + np.exp(-x))


def _bf(a):
    return np.ascontiguousarray(a.astype(ml_dtypes.bfloat16))


def _f32(a):
    return np.ascontiguousarray(a.astype(np.float32))


def _fp8(a):
    return np.ascontiguousarray(
        np.clip(a, -448.0, 448.0).astype(ml_dtypes.float8_e4m3))


def prepare_inputs(x, W_dense, b_dense, tau_n, tau_m, W_ro, b_ro, tau_m_ro):
    x = np.asarray(x, np.float32)
    W = np.asarray(W_dense, np.float32)
    b = np.asarray(b_dense, np.float32)
    beta = _sigmoid(np.asarray(tau_n, np.float32))      # [H, BR]
    alpha = _sigmoid(np.asarray(tau_m, np.float32))     # [H]
    aro = _sigmoid(np.asarray(tau_m_ro, np.float32))    # [O]
    W_ro = np.asarray(W_ro, np.float32)
    b_ro = np.asarray(b_ro, np.float32)

    # branch-major permutation f' = br*H + h  (row f = h*BR + br)
    brs, hs = np.meshgrid(np.arange(BR), np.arange(H), indexing="ij")
    perm = (hs * BR + brs).reshape(-1)
    Wp = W[perm]                                         # [4096, 1724]
    bp = b[perm]
    beta_f = beta.T.reshape(-1)                          # beta[f'=br*H+h]
    alpha_f = np.tile(alpha, BR)                         # alpha[h] per f'
    c = (1.0 - alpha_f) * (1.0 - beta_f)

    Wx = c[:, None] * Wp[:, :IN_DIM]                     # [4096, 700]
    Wh = c[:, None] * Wp[:, IN_DIM:]                     # [4096, 1024]
    bp = c * bp

    Wx_aug = np.zeros((H * BR, KIN), np.float32)
    Wx_aug[:, :IN_DIM] = Wx * WX_SCALE
    Wx_aug[:, IN_DIM] = bp * WX_SCALE                    # bias via x-row == 1

    # lhsT packs: [p, (kt|jhk, jf), m] with lhsT[p, m] = W[jf*128+m, kt*128+p]
    wxT = Wx_aug.reshape(JF, 128, KT, 128).transpose(3, 2, 0, 1).reshape(128, KT * JF * 128)
    whT = Wh.reshape(JF, 128, JH, 128).transpose(3, 2, 0, 1).reshape(128, JH * JF * 128)

    beta_sb = np.repeat(beta_f.reshape(JF, 128).T[:, :, None], BL, axis=2).reshape(128, JF * BL)
    alpha_sb = np.repeat(alpha.reshape(JH, 128).T[:, :, None], BL, axis=2).reshape(128, JH * BL)

    Wrop = (1.0 - aro)[:, None] * W_ro                   # [O, H]
    brop = (1.0 - aro) * b_ro
    wroT = Wrop.reshape(O, JH, 128).transpose(2, 1, 0).reshape(128, JH * O)

    common = {
        "whT": _bf(whT),
        "wxT": _fp8(wxT),
        "beta": _f32(beta_sb),
        "alpha": _f32(alpha_sb),
        "wro": _bf(wroT),
        "bro": _bf(brop.reshape(1, O)),
        "aroc": _f32(aro.reshape(O, 1)),
        "ident": _bf(np.eye(128, dtype=np.float32) / WX_SCALE),
    }

    n_chk = (x.shape[1] * BL + NCH - 1) // NCH
    in_maps = []
    for core in range(NCORES):
        xc = x[core * BL:(core + 1) * BL]                # [32, T, 700]
        t_len = xc.shape[1]
        xT = np.zeros((KIN, (n_chk + PAD_CHK) * NCH), np.float32)
        xT[:IN_DIM, :t_len * BL] = xc.transpose(2, 1, 0).reshape(IN_DIM, t_len * BL)
        xT[IN_DIM, :t_len * BL] = 1.0
        m = dict(common)
        m["xT"] = _bf(xT)
        in_maps.append(m)
    return in_maps


def build_module(t_len=T_FULL, repeat=1, do_pre=True, do_dve=True, do_ident=True, do_ro=True):
    n_chk = (t_len * BL + NCH - 1) // NCH                # 32 for T=500
    tail_steps = t_len - (n_chk - 1) * CH                # steps in last chunk
    assert n_chk >= 6 and (n_chk - 3) % 3 == 0, n_chk

    nc = bacc.Bacc("TRN2", target_bir_lowering=False, debug=False)

    xT = nc.dram_tensor("xT", [KIN, (n_chk + PAD_CHK) * NCH], BF16,
                        kind="ExternalInput").ap()
    whT = nc.dram_tensor("whT", [128, JH * JF * 128], BF16, kind="ExternalInput").ap()
    wxT = nc.dram_tensor("wxT", [128, KT * JF * 128], FP8, kind="ExternalInput").ap()
    beta_in = nc.dram_tensor("beta", [128, JF * BL], F32, kind="ExternalInput").ap()
    alpha_in = nc.dram_tensor("alpha", [128, JH * BL], F32, kind="ExternalInput").ap()
    wro_in = nc.dram_tensor("wro", [128, JH * O], BF16, kind="ExternalInput").ap()
    bro_in = nc.dram_tensor("bro", [1, O], BF16, kind="ExternalInput").ap()
    aroc_in = nc.dram_tensor("aroc", [O, 1], F32, kind="ExternalInput").ap()
    ident_in = nc.dram_tensor("ident", [128, 128], BF16, kind="ExternalInput").ap()
    out = nc.dram_tensor("out", [O, BL], F32, kind="ExternalOutput").ap()
    hist = nc.dram_tensor("hist", [O, n_chk * NCH], BF16).ap()

    with tile.TileContext(nc) as tc:
        _emit(tc, xT, whT, wxT, beta_in, alpha_in, wro_in, bro_in, aroc_in,
              ident_in, out, hist, n_chk=n_chk, tail_steps=tail_steps,
              repeat=repeat, do_pre=do_pre, do_dve=do_dve, do_ident=do_ident,
              do_ro=do_ro)
    nc.compile()
    return nc


def _emit(tc, xT, whT, wxT, beta_in, alpha_in, wro_in, bro_in, aroc_in,
          ident_in, out, hist, n_chk, tail_steps, repeat=1, do_pre=True,
          do_dve=True, do_ident=True, do_ro=True):
    nc = tc.nc

    with (
        tc.tile_pool(name="const", bufs=1) as cpool,
        tc.tile_pool(name="state", bufs=1) as spool,
        tc.tile_pool(name="sm", bufs=1) as smp,
        tc.tile_pool(name="mmps", bufs=6, space="PSUM") as mmps,
        tc.tile_pool(name="prps", bufs=2, space="PSUM") as prps,
    ):
        # ---- resident constants ----
        wx_sb = cpool.tile([128, KT * JF * 128], FP8, tag="wx")
        wh_sb = cpool.tile([128, JH * JF * 128], BF16, tag="wh")
        beta_sb = cpool.tile([128, JF * BL], F32, tag="beta")
        alpha_sb = cpool.tile([128, JH * BL], F32, tag="alpha")
        wro_sb = cpool.tile([128, JH * O], BF16, tag="wro")
        bro_sb = cpool.tile([1, O], BF16, tag="bro")
        aroc_sb = cpool.tile([O, 1], F32, tag="aroc")
        ident = cpool.tile([128, 128], BF16, tag="ident")
        ones20 = cpool.tile([O, 1], F32, tag="ones20")
        onesr = cpool.tile([1, O], F32, tag="onesr")
        ones512 = cpool.tile([1, 512], BF16, tag="ones512")
        arot = cpool.tile([O, (n_chk + PAD_CHK) * CH], F32, tag="arot")
        nc.sync.dma_start(wx_sb[:], wxT[:])
        nc.sync.dma_start(wh_sb[:], whT[:])
        nc.sync.dma_start(beta_sb[:], beta_in[:])
        nc.sync.dma_start(alpha_sb[:], alpha_in[:])
        nc.sync.dma_start(wro_sb[:], wro_in[:])
        nc.sync.dma_start(bro_sb[:], bro_in[:])
        nc.sync.dma_start(aroc_sb[:], aroc_in[:])
        nc.sync.dma_start(ident[:], ident_in[:])
        nc.vector.memset(ones20[:], 1.0)
        nc.vector.memset(onesr[:], 1.0)
        nc.vector.memset(ones512[:], 1.0)
        nc.vector.memset(arot[:], 1.0)
        nc.vector.tensor_scalar(arot[:], arot[:], aroc_sb[:, 0:1], None,
                                mybir.AluOpType.mult)

        # ---- xT staging (2 chunk bufs) + xp chunk bufs (2) ----
        xin_t = spool.tile([128, 3 * KT * NCH], BF16, tag="xin")
        xin = [[xin_t[:, (i * KT + kt) * NCH:(i * KT + kt + 1) * NCH]
                for kt in range(KT)] for i in range(3)]
        xpc_t = spool.tile([128, 3 * CHW], BF16, tag="xpc")
        xpc = [xpc_t[:, i * CHW:(i + 1) * CHW] for i in range(3)]

        # ---- persistent state ----
        d = spool.tile([128, JF * BL], F32, tag="d")
        mem = spool.tile([128, JH * BL], F32, tag="mem")
        spk = [spool.tile([128, SPW], BF16, tag=f"spk{i}", name=f"spk{i}")
               for i in range(2)]
        sphist = [spool.tile([128, CH * SPW], BF16, tag=f"sph{i}", name=f"sph{i}")
                  for i in range(3)]
        mp = spool.tile([128, JH * BL], F32, tag="mp")
        histc = [spool.tile([O, NCH], BF16, tag=f"hc{i}", name=f"hc{i}")
                 for i in range(3)]
        accT = spool.tile([O, BL], F32, tag="accT")
        l_t1 = spool.tile([128, 4 * BL], F32, tag="lt1")
        l_t2 = spool.tile([128, 4 * BL], F32, tag="lt2")
        l_half = spool.tile([128, 4 * BL], F32, tag="lh")

        d_v = d[:].rearrange("p (br c) -> p br c", br=BR)
        beta_v = beta_sb[:].rearrange("p (br c) -> p br c", br=BR)

        # ================= phase-1 producer: one chunk of xp =================
        def load_x(buf, coff):
            for kt in range(KT):
                src = (xT[kt * 128:(kt + 1) * 128, bass.ds(coff, NCH)]
                       if not isinstance(coff, int)
                       else xT[kt * 128:(kt + 1) * 128, coff:coff + NCH])
                nc.sync.dma_start(xin[buf][kt], src)

        def pre_group(buf, jf):
            if not do_pre:
                return
            ps = prps.tile([128, NCH], F32, tag="pre")
            for kt in range(KT):
                nc.tensor.matmul(
                    ps[:],
                    wx_sb[:, (kt * JF + jf) * 128:(kt * JF + jf + 1) * 128],
                    xin[buf][kt],
                    start=(kt == 0), stop=(kt == KT - 1),
                )
            nc.scalar.copy(xpc[buf][:, jf * NCH:(jf + 1) * NCH], ps[:])

        def pre_chunk(buf):
            for jf in range(JF):
                pre_group(buf, jf)

        # ================= scan consumer =================
        def emit_step(buf, s, cmod):
            if s == 0:
                sp_prev = sphist[(cmod + 2) % 3][:, (CH - 1) * SPW:CH * SPW]
            else:
                sp_prev = sphist[cmod][:, (s - 1) * SPW:s * SPW]
            sp_cur = sphist[cmod][:, s * SPW:(s + 1) * SPW]
            xpc_v = xpc[buf].rearrange("p (br q s b) -> p br q s b",
                                       br=BR, q=JH, s=CH)
            if do_dve:
                # off-chain prep on the otherwise-idle Pool engine: these only
                # need step t-1 state and have a full PE-window of slack, so
                # GPSIMD's lower throughput is hidden while DVE sheds ~1.5us
                nc.gpsimd.tensor_tensor(d[:], d[:], beta_sb[:],
                                        mybir.AluOpType.mult)
                nc.gpsimd.tensor_tensor(mp[:], mem[:], alpha_sb[:],
                                        mybir.AluOpType.mult)
                nc.gpsimd.tensor_tensor(mp[:], mp[:], sp_prev,
                                        mybir.AluOpType.subtract)
            hps = []
            for half in (0, 1):
                hp = mmps.tile([128, 512], F32, tag="mm")
                if do_ident:
                    xsl = xpc_v[:, :, half * 4:half * 4 + 4, s, :]
                    nc.tensor.matmul(hp[:], ident[:], xsl, start=True, stop=False,
                                     skip_group_check=True)
                for bri in range(BR):
                    for jho in range(4):
                        jf = bri * 8 + half * 4 + jho
                        o_sl = hp[:, bri * 128 + jho * 32: bri * 128 + jho * 32 + 32]
                        for jhk in range(JH):
                            nc.tensor.matmul(
                                o_sl,
                                wh_sb[:, (jhk * JF + jf) * 128:(jhk * JF + jf + 1) * 128],
                                sp_prev[:, jhk * 32:jhk * 32 + 32],
                                start=(not do_ident and jhk == 0),
                                stop=(jhk == JH - 1),
                                skip_group_check=True,
                            )
                hps.append(hp)

            for half in (0, 1):
                if not do_dve:
                    break
                hp = hps[half]
                off = half * 128
                dsl = d_v[:, :, off:off + 128]
                nc.vector.tensor_tensor(dsl, dsl, hp[:], mybir.AluOpType.add)
                nc.vector.tensor_tensor(l_t1[:], d_v[:, 0, off:off + 128],
                                        d_v[:, 1, off:off + 128], mybir.AluOpType.add)
                nc.vector.tensor_tensor(l_t2[:], d_v[:, 2, off:off + 128],
                                        d_v[:, 3, off:off + 128], mybir.AluOpType.add)
                nc.vector.tensor_tensor(l_half[:], l_t1[:], l_t2[:],
                                        mybir.AluOpType.add)
                msl = mem[:, off:off + 128]
                nc.vector.tensor_tensor(msl, mp[:, off:off + 128], l_half[:],
                                        mybir.AluOpType.add)
                nc.vector.tensor_scalar(sp_cur[:, off:off + 128], msl, 1.0, None,
                                        mybir.AluOpType.is_gt)

        def chunk_readout(cmod):
            if not do_ro:
                return
            sp_v = sphist[cmod][:].rearrange("p (s q b) -> p s q b", s=CH, q=JH)
            roT = mmps.tile([O, NCH], F32, tag="mm")
            for jh in range(JH):
                nc.tensor.matmul(
                    roT[:], wro_sb[:, jh * O:(jh + 1) * O],
                    sp_v[:, :, jh, :],
                    start=(jh == 0), stop=False,
                )
            nc.tensor.matmul(roT[:], bro_sb[:], ones512[:, :NCH], start=False,
                             stop=True)
            nc.scalar.copy(histc[cmod][:], roT[:])

        def flush_hist(cmod, xoff):
            if not do_ro:
                return
            dst = (hist[:, bass.ds(xoff, NCH)] if not isinstance(xoff, int)
                   else hist[:, xoff:xoff + NCH])
            nc.scalar.dma_start(dst, histc[cmod][:])

        for _rep in range(repeat):
            # ---- reset state ----
            nc.vector.memset(d[:], 0.0)
            nc.vector.memset(mem[:], 0.0)
            nc.vector.memset(spk[0][:], 0.0)
            nc.vector.memset(spk[1][:], 0.0)
            for sph in sphist:
                nc.vector.memset(sph[:], 0.0)
            if not do_pre:
                nc.vector.memset(xpc_t[:], 0.0)
            if not do_dve:
                nc.vector.memset(mem[:], 0.0)
            nc.vector.memset(accT[:], 0.0)

            # ---- fused pipeline: produce chunk c+2 (buf (c+2)%3) while
            # scanning chunk c (buf c%3); 2 chunks of pipeline slack.
            GPS = JF // CH                               # pre groups per step (4)

            def scan_chunk(c_mod, prod_buf, ho, lo_chunk=None, nsteps=CH):
                for s in range(nsteps):
                    emit_step(c_mod, s, c_mod)
                    if prod_buf is not None:
                        for g in range(GPS):
                            pre_group(prod_buf, GPS * s + g)
                chunk_readout(c_mod)
                flush_hist(c_mod, ho)

            load_x(0, 0)
            pre_chunk(0)
            load_x(1, NCH)
            pre_chunk(1)

            # chunks 0..n_loop*3-1 in the loop; n_chk = 63 -> loop 20 iters
            # covers 0..59, then 60, 61, 62 peeled.
            n_loop = (n_chk - 3) // 3
            with tc.For_i(0, n_loop * 3 * NCH, 3 * NCH,
                          hint_engines=(mybir.EngineType.PE,)) as xoff:
                load_x(2, xoff + 2 * NCH)
                scan_chunk(0, 2, xoff)
                load_x(0, xoff + 3 * NCH)
                scan_chunk(1, 0, xoff + NCH)
                load_x(1, xoff + 4 * NCH)
                scan_chunk(2, 1, xoff + 2 * NCH)

            base = n_loop * 3
            # remaining chunks: base, base+1, base+2 (= n_chk-1, tail)
            load_x(2, (base + 2) * NCH)
            scan_chunk(base % 3, 2, base * NCH)
            scan_chunk((base + 1) % 3, None, (base + 1) * NCH)
            for s in range(tail_steps):
                emit_step((base + 2) % 3, s, (base + 2) % 3)
            chunk_readout((base + 2) % 3)
            flush_hist((base + 2) % 3, (base + 2) * NCH)

            # ---- end-pass ----
            hist_sb = xpc_t[0:O, 0:n_chk * NCH]
            nc.sync.dma_start(hist_sb, hist[:])
            hv = hist_sb.rearrange("p (t b) -> p t b", b=BL)
            seqs = hist_sb                               # in-place filter
            sv = hv
            for b in range(BL):
                nc.vector.tensor_tensor_scan(
                    sv[:, :, b], arot[:, 0:n_chk * CH], hv[:, :, b], 0.0,
                    mybir.AluOpType.mult, mybir.AluOpType.add)
            for c in range(n_chk):
                t0 = c * CH
                lo = max(WARMUP + 1 - t0, 0)
                hi = tail_steps if c == n_chk - 1 else CH
                if lo >= hi:
                    continue
                sq = seqs[:, c * NCH:(c + 1) * NCH]
                e = smp.tile([O, NCH], F32, tag="e")
                nc.scalar.activation(e[:], sq, mybir.ActivationFunctionType.Exp)
                sp = mmps.tile([1, 512], F32, tag="mm")
                nc.tensor.matmul(sp[:, :NCH], ones20[:], e[:], start=True,
                                 stop=True)
                rp = smp.tile([1, NCH], F32, tag="rp")
                nc.vector.reciprocal(rp[:], sp[:, :NCH])
                bc = mmps.tile([128, 512], F32, tag="mm")
                nc.tensor.matmul(bc[:O, :NCH], onesr[:], rp[:], start=True,
                                 stop=True)
                pr = smp.tile([O, NCH], F32, tag="pr")
                nc.vector.tensor_tensor(pr[:], e[:], bc[:O, :NCH],
                                        mybir.AluOpType.mult)
                pr_v = pr[:].rearrange("p (s b) -> p b s", s=CH)
                red = smp.tile([O, BL], F32, tag="red")
                nc.vector.tensor_reduce(red[:], pr_v[:, :, lo:hi],
                                        mybir.AxisListType.X,
                                        mybir.AluOpType.add)
                nc.vector.tensor_tensor(accT[:], accT[:], red[:],
                                        mybir.AluOpType.add)

            nc.sync.dma_start(out[:], accT[:])


_NC_CACHE = {}


def _get_module(t_len):
    if t_len not in _NC_CACHE:
        _NC_CACHE[t_len] = build_module(t_len)
    return _NC_CACHE[t_len]


def run(inputs, trace=False):
    in_maps = prepare_inputs(**inputs)
    t_len = np.asarray(inputs["x"]).shape[1]
    nc = _get_module(t_len)
    res = run_bass_kernel_spmd(nc, in_maps, list(range(NCORES)), trace=trace)
    outs = [res.results[i]["out"].T for i in range(NCORES)]   # [O,BL] -> [BL,O]
    return np.concatenate(outs, axis=0).astype(np.float32), res


def kernel(x, W_dense, b_dense, tau_n, tau_m, W_ro, b_ro, tau_m_ro):
    out, _ = run(dict(x=x, W_dense=W_dense, b_dense=b_dense, tau_n=tau_n,
                      tau_m=tau_m, W_ro=W_ro, b_ro=b_ro, tau_m_ro=tau_m_ro))
    return out


def make_bench(inputs, nc=None, prep_kwargs=None):
    """Build a timed runner with device-resident inputs (for test.py only).

    Mirrors bass2jax.run_bass_via_pjrt's multi-core path, but device_puts the
    inputs once so repeated calls measure device execution, not host transfer.
    """
    import jax
    import numpy as np_
    from jax.sharding import Mesh, PartitionSpec, NamedSharding
    from jax.experimental.shard_map import shard_map
    import concourse.mybir as mybir_
    from concourse import bass2jax

    in_maps = prepare_inputs(**inputs, **(prep_kwargs or {}))
    t_len = np_.asarray(inputs["x"]).shape[1]
    if nc is None:
        nc = _get_module(t_len)
    bass2jax.install_neuronx_cc_hook()

    partition_name = nc.partition_id_tensor.name if nc.partition_id_tensor else None
    in_names, out_names, out_avals, zero_outs = [], [], [], []
    for alloc in nc.m.functions[0].allocations:
        if not isinstance(alloc, mybir_.MemoryLocationSet):
            continue
        name = alloc.memorylocations[0].name
        if alloc.kind == "ExternalInput":
            if name != partition_name:
                in_names.append(name)
        elif alloc.kind == "ExternalOutput":
            shape = tuple(alloc.tensor_shape)
            dtype = mybir_.dt.np(alloc.dtype)
            out_names.append(name)
            out_avals.append(jax.core.ShapedArray(shape, dtype))
            zero_outs.append(np_.zeros(shape, dtype))
    n_params = len(in_names)
    all_in_names = in_names + out_names
    if partition_name is not None:
        all_in_names.append(partition_name)
    donate = tuple(range(n_params, n_params + len(out_names)))

    def _body(*args):
        operands = list(args)
        if partition_name is not None:
            operands.append(bass2jax.partition_id_tensor())
        outs = bass2jax._bass_exec_p.bind(
            *operands,
            out_avals=tuple(out_avals),
            in_names=tuple(all_in_names),
            out_names=tuple(out_names),
            lowering_input_output_aliases=(),
            sim_require_finite=True,
            sim_require_nnan=True,
            nc=nc,
        )
        return tuple(outs)

    devices = jax.devices()[:NCORES]
    mesh = Mesh(np_.asarray(devices), ("core",))
    in_specs = (PartitionSpec("core"),) * (n_params + len(out_names))
    out_specs = (PartitionSpec("core"),) * len(out_names)
    sharded = jax.jit(
        shard_map(_body, mesh=mesh, in_specs=in_specs, out_specs=out_specs,
                  check_rep=False),
        donate_argnums=donate, keep_unused=True,
    )
    concat_in = [
        np_.concatenate([np_.asarray(in_maps[c][name]) for c in range(NCORES)], axis=0)
        for name in in_names
    ]
    sh = NamedSharding(mesh, PartitionSpec("core"))
    dev_in = [jax.device_put(a, sh) for a in concat_in]

    def call():
        zeros = [np_.zeros((NCORES * z.shape[0], *z.shape[1:]), z.dtype)
                 for z in zero_outs]
        outs = sharded(*dev_in, *zeros)
        jax.block_until_ready(outs)
        return outs

    return call



# revision 5
# speedup vs baseline: 1.5118x; 1.5118x over previous
"""DH-SRNN forward on 8 Trainium2 cores — fused pre/scan pipeline.

Data-parallel over batch (B=256 -> 32 rows/core), weights replicated.
Measured ~5.8-6.9 ms vs the 8.8 ms two-phase baseline. Key structure:
  - Wh in bf16: the 256 scan matmuls/step run at ~31ns/MM (vs 46ns fp8; the
    stationary-load path is faster for bf16, and fp8 DoubleRow loses at n=32).
  - Phase-1 (xp = x @ Wx'^T) is fused INTO the scan loop as a
    producer-consumer pipeline over 2 SBUF chunk buffers: chunk c+2's GEMM
    fills PE idle slots while chunk c's scan runs, and xp never touches DRAM.
  - xp enters d via PSUM preload (identity matmul, start=True) so the DVE
    d-update is 2 passes (beta-mult, psum-add) instead of 3.
  - Spikes for a chunk are kept in a history tile; the readout GEMM runs
    once per chunk with n=512 (8+1 matmuls) instead of per step with n=32.
  - The readout membrane filter (mem_ro = aro*mem_ro + roT) moved off the
    scan entirely: raw roT history goes to DRAM; the end-pass applies the
    exponential filter with tensor_tensor_scan along time (stride-32 view,
    in place), then softmax + warmup-masked sum.
  - Pipeline: CH=8-step chunks, 3 SBUF chunk buffers, chunk c+2 produced
    (4 pre jf-groups interleaved after each scan step, explicitly in program
    order - the tile scheduler does NOT fill PE gaps on its own) while chunk
    c is scanned; 2 chunks of slack hide the PSUM->SBUF evacuation latency.
  - PSUM: 6 banks for the scan's hp tiles (rotation depth 3 steps; fewer
    banks stall the PE), 2 for the pre GEMM.
  - mem update split: mp = alpha*mem - spk computed off-chain; on-chain
    mem = mp + l_half. d *= beta also runs off-chain before hp lands.

Math (host-side fold, exact):
  beta = sigmoid(tau_n)[H,BR], alpha = sigmoid(tau_m)[H], aro = sigmoid(tau_m_ro)
  f' = br*H + h (branch-major), c[f'] = (1-alpha[h])*(1-beta[h,br]) folded
  into W_dense rows/bias; D := (1-alpha)*d
     D_t   = beta*D_{t-1} + (xp'_t + spk_{t-1} @ Wh'^T)
     mem_t = alpha*mem_{t-1} + sum_br D_t - spk_{t-1}
     spk_t = (mem_t > 1)
     roT_t = Wro' @ spk_t + bro'        (Wro' = (1-aro)*W_ro etc.)
  end:  mem_ro_t = aro*mem_ro_{t-1} + roT_t ;  out = sum_{t>10} softmax(mem_ro_t)
"""

import numpy as np
import ml_dtypes

import concourse.bass as bass
import concourse.bacc as bacc
import concourse.mybir as mybir
import concourse.tile as tile
from concourse.bass_utils import run_bass_kernel_spmd

F32 = mybir.dt.float32
BF16 = mybir.dt.bfloat16
FP8 = mybir.dt.float8e4

B, T_FULL, IN_DIM = 256, 500, 700
H, BR, O = 1024, 4, 20
NCORES = 8
BL = B // NCORES            # 32 batch rows per core
KT = 6                      # k-tiles for input dim (700 + bias row -> 768)
KIN = KT * 128              # 768
JF = (H * BR) // 128        # 32 feature blocks
JH = H // 128               # 8 hidden blocks
CH = 8                      # timesteps per chunk
NCH = CH * BL               # 512 xT cols per chunk
CHW = JF * NCH              # xp cols per chunk (16384)
SPW = JH * BL               # spike cols per step (256)
PAD_CHK = 2                 # extra xT chunks so the producer can overrun
WARMUP = 10
WX_SCALE = 1024.0           # lifts Wx into fp8e4m3 normal range; undone by
                            # ident = eye/WX_SCALE in the xp-preload matmul


def _sigmoid(x):
    return 1.0 / (1.0 + np.exp(-x))


def _bf(a):
    return np.ascontiguousarray(a.astype(ml_dtypes.bfloat16))


def _f32(a):
    return np.ascontiguousarray(a.astype(np.float32))


def _fp8(a):
    return np.ascontiguousarray(
        np.clip(a, -448.0, 448.0).astype(ml_dtypes.float8_e4m3))


def prepare_inputs(x, W_dense, b_dense, tau_n, tau_m, W_ro, b_ro, tau_m_ro):
    x = np.asarray(x, np.float32)
    W = np.asarray(W_dense, np.float32)
    b = np.asarray(b_dense, np.float32)
    beta = _sigmoid(np.asarray(tau_n, np.float32))      # [H, BR]
    alpha = _sigmoid(np.asarray(tau_m, np.float32))     # [H]
    aro = _sigmoid(np.asarray(tau_m_ro, np.float32))    # [O]
    W_ro = np.asarray(W_ro, np.float32)
    b_ro = np.asarray(b_ro, np.float32)

    # branch-major permutation f' = br*H + h  (row f = h*BR + br)
    brs, hs = np.meshgrid(np.arange(BR), np.arange(H), indexing="ij")
    perm = (hs * BR + brs).reshape(-1)
    Wp = W[perm]                                         # [4096, 1724]
    bp = b[perm]
    beta_f = beta.T.reshape(-1)                          # beta[f'=br*H+h]
    alpha_f = np.tile(alpha, BR)                         # alpha[h] per f'
    c = (1.0 - alpha_f) * (1.0 - beta_f)

    Wx = c[:, None] * Wp[:, :IN_DIM]                     # [4096, 700]
    Wh = c[:, None] * Wp[:, IN_DIM:]                     # [4096, 1024]
    bp = c * bp

    Wx_aug = np.zeros((H * BR, KIN), np.float32)
    Wx_aug[:, :IN_DIM] = Wx * WX_SCALE
    Wx_aug[:, IN_DIM] = bp * WX_SCALE                    # bias via x-row == 1

    # lhsT packs: [p, (kt|jhk, jf), m] with lhsT[p, m] = W[jf*128+m, kt*128+p]
    wxT = Wx_aug.reshape(JF, 128, KT, 128).transpose(3, 2, 0, 1).reshape(128, KT * JF * 128)
    whT = Wh.reshape(JF, 128, JH, 128).transpose(3, 2, 0, 1).reshape(128, JH * JF * 128)

    beta_sb = np.repeat(beta_f.reshape(JF, 128).T[:, :, None], BL, axis=2).reshape(128, JF * BL)
    alpha_sb = np.repeat(alpha.reshape(JH, 128).T[:, :, None], BL, axis=2).reshape(128, JH * BL)

    Wrop = (1.0 - aro)[:, None] * W_ro                   # [O, H]
    brop = (1.0 - aro) * b_ro
    wroT = Wrop.reshape(O, JH, 128).transpose(2, 1, 0).reshape(128, JH * O)

    common = {
        "whT": _bf(whT),
        "wxT": _fp8(wxT),
        "beta": _f32(beta_sb),
        "alpha": _f32(alpha_sb),
        "wro": _bf(wroT),
        "bro": _bf(brop.reshape(1, O)),
        "aroc": _f32(aro.reshape(O, 1)),
        "ident": _bf(np.eye(128, dtype=np.float32) / WX_SCALE),
    }

    n_chk = (x.shape[1] * BL + NCH - 1) // NCH
    in_maps = []
    for core in range(NCORES):
        xc = x[core * BL:(core + 1) * BL]                # [32, T, 700]
        t_len = xc.shape[1]
        xT = np.zeros((KIN, (n_chk + PAD_CHK) * NCH), np.float32)
        xT[:IN_DIM, :t_len * BL] = xc.transpose(2, 1, 0).reshape(IN_DIM, t_len * BL)
        xT[IN_DIM, :t_len * BL] = 1.0
        m = dict(common)
        m["xT"] = _bf(xT)
        in_maps.append(m)
    return in_maps


def build_module(t_len=T_FULL, repeat=1, do_pre=True, do_dve=True, do_ident=True, do_ro=True):
    n_chk = (t_len * BL + NCH - 1) // NCH                # 32 for T=500
    tail_steps = t_len - (n_chk - 1) * CH                # steps in last chunk
    assert n_chk >= 6 and (n_chk - 3) % 3 == 0, n_chk

    nc = bacc.Bacc("TRN2", target_bir_lowering=False, debug=False)

    xT = nc.dram_tensor("xT", [KIN, (n_chk + PAD_CHK) * NCH], BF16,
                        kind="ExternalInput").ap()
    whT = nc.dram_tensor("whT", [128, JH * JF * 128], BF16, kind="ExternalInput").ap()
    wxT = nc.dram_tensor("wxT", [128, KT * JF * 128], FP8, kind="ExternalInput").ap()
    beta_in = nc.dram_tensor("beta", [128, JF * BL], F32, kind="ExternalInput").ap()
    alpha_in = nc.dram_tensor("alpha", [128, JH * BL], F32, kind="ExternalInput").ap()
    wro_in = nc.dram_tensor("wro", [128, JH * O], BF16, kind="ExternalInput").ap()
    bro_in = nc.dram_tensor("bro", [1, O], BF16, kind="ExternalInput").ap()
    aroc_in = nc.dram_tensor("aroc", [O, 1], F32, kind="ExternalInput").ap()
    ident_in = nc.dram_tensor("ident", [128, 128], BF16, kind="ExternalInput").ap()
    out = nc.dram_tensor("out", [O, BL], F32, kind="ExternalOutput").ap()
    hist = nc.dram_tensor("hist", [O, n_chk * NCH], BF16).ap()

    with tile.TileContext(nc) as tc:
        _emit(tc, xT, whT, wxT, beta_in, alpha_in, wro_in, bro_in, aroc_in,
              ident_in, out, hist, n_chk=n_chk, tail_steps=tail_steps,
              repeat=repeat, do_pre=do_pre, do_dve=do_dve, do_ident=do_ident,
              do_ro=do_ro)
    nc.compile()
    return nc


def _emit(tc, xT, whT, wxT, beta_in, alpha_in, wro_in, bro_in, aroc_in,
          ident_in, out, hist, n_chk, tail_steps, repeat=1, do_pre=True,
          do_dve=True, do_ident=True, do_ro=True):
    nc = tc.nc

    with (
        tc.tile_pool(name="const", bufs=1) as cpool,
        tc.tile_pool(name="state", bufs=1) as spool,
        tc.tile_pool(name="sm", bufs=1) as smp,
        tc.tile_pool(name="mmps", bufs=6, space="PSUM") as mmps,
        tc.tile_pool(name="prps", bufs=2, space="PSUM") as prps,
    ):
        # ---- resident constants ----
        wx_sb = cpool.tile([128, KT * JF * 128], FP8, tag="wx")
        wh_sb = cpool.tile([128, JH * JF * 128], BF16, tag="wh")
        beta_sb = cpool.tile([128, JF * BL], F32, tag="beta")
        alpha_sb = cpool.tile([128, JH * BL], F32, tag="alpha")
        wro_sb = cpool.tile([128, JH * O], BF16, tag="wro")
        bro_sb = cpool.tile([1, O], BF16, tag="bro")
        aroc_sb = cpool.tile([O, 1], F32, tag="aroc")
        ident = cpool.tile([128, 128], BF16, tag="ident")
        ones20 = cpool.tile([O, 1], F32, tag="ones20")
        onesr = cpool.tile([1, O], F32, tag="onesr")
        ones512 = cpool.tile([1, 512], BF16, tag="ones512")
        arot = cpool.tile([O, (n_chk + PAD_CHK) * CH], F32, tag="arot")
        nc.sync.dma_start(wx_sb[:], wxT[:])
        nc.sync.dma_start(wh_sb[:], whT[:])
        nc.sync.dma_start(beta_sb[:], beta_in[:])
        nc.sync.dma_start(alpha_sb[:], alpha_in[:])
        nc.sync.dma_start(wro_sb[:], wro_in[:])
        nc.sync.dma_start(bro_sb[:], bro_in[:])
        nc.sync.dma_start(aroc_sb[:], aroc_in[:])
        nc.sync.dma_start(ident[:], ident_in[:])
        nc.vector.memset(ones20[:], 1.0)
        nc.vector.memset(onesr[:], 1.0)
        nc.vector.memset(ones512[:], 1.0)
        nc.vector.memset(arot[:], 1.0)
        nc.vector.tensor_scalar(arot[:], arot[:], aroc_sb[:, 0:1], None,
                                mybir.AluOpType.mult)

        # ---- xT staging (2 chunk bufs) + xp chunk bufs (2) ----
        xin_t = spool.tile([128, 3 * KT * NCH], BF16, tag="xin")
        xin = [[xin_t[:, (i * KT + kt) * NCH:(i * KT + kt + 1) * NCH]
                for kt in range(KT)] for i in range(3)]
        xpc_t = spool.tile([128, 3 * CHW], BF16, tag="xpc")
        xpc = [xpc_t[:, i * CHW:(i + 1) * CHW] for i in range(3)]

        # ---- persistent state ----
        d = spool.tile([128, JF * BL], F32, tag="d")
        mem = spool.tile([128, JH * BL], F32, tag="mem")
        spk = [spool.tile([128, SPW], BF16, tag=f"spk{i}", name=f"spk{i}")
               for i in range(2)]
        sphist = [spool.tile([128, CH * SPW], BF16, tag=f"sph{i}", name=f"sph{i}")
                  for i in range(3)]
        mp = spool.tile([128, JH * BL], F32, tag="mp")
        histc = [spool.tile([O, NCH], BF16, tag=f"hc{i}", name=f"hc{i}")
                 for i in range(3)]
        accT = spool.tile([O, BL], F32, tag="accT")
        l_t1 = spool.tile([128, 4 * BL], F32, tag="lt1")
        l_t2 = spool.tile([128, 4 * BL], F32, tag="lt2")
        l_half = spool.tile([128, 4 * BL], F32, tag="lh")

        d_v = d[:].rearrange("p (br c) -> p br c", br=BR)
        beta_v = beta_sb[:].rearrange("p (br c) -> p br c", br=BR)

        # ================= phase-1 producer: one chunk of xp =================
        def load_x(buf, coff):
            for kt in range(KT):
                src = (xT[kt * 128:(kt + 1) * 128, bass.ds(coff, NCH)]
                       if not isinstance(coff, int)
                       else xT[kt * 128:(kt + 1) * 128, coff:coff + NCH])
                nc.sync.dma_start(xin[buf][kt], src)

        def pre_group(buf, jf):
            if not do_pre:
                return
            ps = prps.tile([128, NCH], F32, tag="pre")
            for kt in range(KT):
                nc.tensor.matmul(
                    ps[:],
                    wx_sb[:, (kt * JF + jf) * 128:(kt * JF + jf + 1) * 128],
                    xin[buf][kt],
                    start=(kt == 0), stop=(kt == KT - 1),
                )
            nc.scalar.copy(xpc[buf][:, jf * NCH:(jf + 1) * NCH], ps[:])

        def pre_chunk(buf):
            for jf in range(JF):
                pre_group(buf, jf)

        # ================= scan consumer =================
        def emit_step(buf, s, cmod):
            if s == 0:
                sp_prev = sphist[(cmod + 2) % 3][:, (CH - 1) * SPW:CH * SPW]
            else:
                sp_prev = sphist[cmod][:, (s - 1) * SPW:s * SPW]
            sp_cur = sphist[cmod][:, s * SPW:(s + 1) * SPW]
            xpc_v = xpc[buf].rearrange("p (br q s b) -> p br q s b",
                                       br=BR, q=JH, s=CH)
            if do_dve:
                # off-chain prep: d *= beta and mp = alpha*mem - spk_prev can
                # run on DVE while the PE is still producing hp
                nc.vector.tensor_tensor(d[:], d[:], beta_sb[:],
                                        mybir.AluOpType.mult)
                nc.vector.tensor_tensor(mp[:], mem[:], alpha_sb[:],
                                        mybir.AluOpType.mult)
                nc.vector.tensor_tensor(mp[:], mp[:], sp_prev,
                                        mybir.AluOpType.subtract)
            hps = []
            for half in (0, 1):
                hp = mmps.tile([128, 512], F32, tag="mm")
                if do_ident:
                    xsl = xpc_v[:, :, half * 4:half * 4 + 4, s, :]
                    nc.tensor.matmul(hp[:], ident[:], xsl, start=True, stop=False,
                                     skip_group_check=True)
                for bri in range(BR):
                    for jho in range(4):
                        jf = bri * 8 + half * 4 + jho
                        o_sl = hp[:, bri * 128 + jho * 32: bri * 128 + jho * 32 + 32]
                        for jhk in range(JH):
                            nc.tensor.matmul(
                                o_sl,
                                wh_sb[:, (jhk * JF + jf) * 128:(jhk * JF + jf + 1) * 128],
                                sp_prev[:, jhk * 32:jhk * 32 + 32],
                                start=(not do_ident and jhk == 0),
                                stop=(jhk == JH - 1),
                                skip_group_check=True,
                            )
                hps.append(hp)

            for half in (0, 1):
                if not do_dve:
                    break
                hp = hps[half]
                off = half * 128
                dsl = d_v[:, :, off:off + 128]
                nc.vector.tensor_tensor(dsl, dsl, hp[:], mybir.AluOpType.add)
                nc.vector.tensor_tensor(l_t1[:], d_v[:, 0, off:off + 128],
                                        d_v[:, 1, off:off + 128], mybir.AluOpType.add)
                nc.vector.tensor_tensor(l_t2[:], d_v[:, 2, off:off + 128],
                                        d_v[:, 3, off:off + 128], mybir.AluOpType.add)
                nc.vector.tensor_tensor(l_half[:], l_t1[:], l_t2[:],
                                        mybir.AluOpType.add)
                msl = mem[:, off:off + 128]
                nc.vector.tensor_tensor(msl, mp[:, off:off + 128], l_half[:],
                                        mybir.AluOpType.add)
                nc.vector.tensor_scalar(sp_cur[:, off:off + 128], msl, 1.0, None,
                                        mybir.AluOpType.is_gt)

        def chunk_readout(cmod):
            if not do_ro:
                return
            sp_v = sphist[cmod][:].rearrange("p (s q b) -> p s q b", s=CH, q=JH)
            roT = mmps.tile([O, NCH], F32, tag="mm")
            for jh in range(JH):
                nc.tensor.matmul(
                    roT[:], wro_sb[:, jh * O:(jh + 1) * O],
                    sp_v[:, :, jh, :],
                    start=(jh == 0), stop=False,
                )
            nc.tensor.matmul(roT[:], bro_sb[:], ones512[:, :NCH], start=False,
                             stop=True)
            nc.scalar.copy(histc[cmod][:], roT[:])

        def flush_hist(cmod, xoff):
            if not do_ro:
                return
            dst = (hist[:, bass.ds(xoff, NCH)] if not isinstance(xoff, int)
                   else hist[:, xoff:xoff + NCH])
            nc.scalar.dma_start(dst, histc[cmod][:])

        for _rep in range(repeat):
            # ---- reset state ----
            nc.vector.memset(d[:], 0.0)
            nc.vector.memset(mem[:], 0.0)
            nc.vector.memset(spk[0][:], 0.0)
            nc.vector.memset(spk[1][:], 0.0)
            for sph in sphist:
                nc.vector.memset(sph[:], 0.0)
            if not do_pre:
                nc.vector.memset(xpc_t[:], 0.0)
            if not do_dve:
                nc.vector.memset(mem[:], 0.0)
            nc.vector.memset(accT[:], 0.0)

            # ---- fused pipeline: produce chunk c+2 (buf (c+2)%3) while
            # scanning chunk c (buf c%3); 2 chunks of pipeline slack.
            GPS = JF // CH                               # pre groups per step (4)

            def scan_chunk(c_mod, prod_buf, ho, lo_chunk=None, nsteps=CH):
                for s in range(nsteps):
                    emit_step(c_mod, s, c_mod)
                    if prod_buf is not None:
                        for g in range(GPS):
                            pre_group(prod_buf, GPS * s + g)
                chunk_readout(c_mod)
                flush_hist(c_mod, ho)

            load_x(0, 0)
            pre_chunk(0)
            load_x(1, NCH)
            pre_chunk(1)

            # chunks 0..n_loop*3-1 in the loop; n_chk = 63 -> loop 20 iters
            # covers 0..59, then 60, 61, 62 peeled.
            n_loop = (n_chk - 3) // 3
            with tc.For_i(0, n_loop * 3 * NCH, 3 * NCH,
                          hint_engines=(mybir.EngineType.PE,)) as xoff:
                load_x(2, xoff + 2 * NCH)
                scan_chunk(0, 2, xoff)
                load_x(0, xoff + 3 * NCH)
                scan_chunk(1, 0, xoff + NCH)
                load_x(1, xoff + 4 * NCH)
                scan_chunk(2, 1, xoff + 2 * NCH)

            base = n_loop * 3
            # remaining chunks: base, base+1, base+2 (= n_chk-1, tail)
            load_x(2, (base + 2) * NCH)
            scan_chunk(base % 3, 2, base * NCH)
            scan_chunk((base + 1) % 3, None, (base + 1) * NCH)
            for s in range(tail_steps):
                emit_step((base + 2) % 3, s, (base + 2) % 3)
            chunk_readout((base + 2) % 3)
            flush_hist((base + 2) % 3, (base + 2) * NCH)

            # ---- end-pass ----
            hist_sb = xpc_t[0:O, 0:n_chk * NCH]
            nc.sync.dma_start(hist_sb, hist[:])
            hv = hist_sb.rearrange("p (t b) -> p t b", b=BL)
            seqs = hist_sb                               # in-place filter
            sv = hv
            for b in range(BL):
                nc.vector.tensor_tensor_scan(
                    sv[:, :, b], arot[:, 0:n_chk * CH], hv[:, :, b], 0.0,
                    mybir.AluOpType.mult, mybir.AluOpType.add)
            for c in range(n_chk):
                t0 = c * CH
                lo = max(WARMUP + 1 - t0, 0)
                hi = tail_steps if c == n_chk - 1 else CH
                if lo >= hi:
                    continue
                sq = seqs[:, c * NCH:(c + 1) * NCH]
                e = smp.tile([O, NCH], F32, tag="e")
                nc.scalar.activation(e[:], sq, mybir.ActivationFunctionType.Exp)
                sp = mmps.tile([1, 512], F32, tag="mm")
                nc.tensor.matmul(sp[:, :NCH], ones20[:], e[:], start=True,
                                 stop=True)
                rp = smp.tile([1, NCH], F32, tag="rp")
                nc.vector.reciprocal(rp[:], sp[:, :NCH])
                bc = mmps.tile([128, 512], F32, tag="mm")
                nc.tensor.matmul(bc[:O, :NCH], onesr[:], rp[:], start=True,
                                 stop=True)
                pr = smp.tile([O, NCH], F32, tag="pr")
                nc.vector.tensor_tensor(pr[:], e[:], bc[:O, :NCH],
                                        mybir.AluOpType.mult)
                pr_v = pr[:].rearrange("p (s b) -> p b s", s=CH)
                red = smp.tile([O, BL], F32, tag="red")
                nc.vector.tensor_reduce(red[:], pr_v[:, :, lo:hi],
                                        mybir.AxisListType.X,
                                        mybir.AluOpType.add)
                nc.vector.tensor_tensor(accT[:], accT[:], red[:],
                                        mybir.AluOpType.add)

            nc.sync.dma_start(out[:], accT[:])


_NC_CACHE = {}


def _get_module(t_len):
    if t_len not in _NC_CACHE:
        _NC_CACHE[t_len] = build_module(t_len)
    return _NC_CACHE[t_len]


def run(inputs, trace=False):
    in_maps = prepare_inputs(**inputs)
    t_len = np.asarray(inputs["x"]).shape[1]
    nc = _get_module(t_len)
    res = run_bass_kernel_spmd(nc, in_maps, list(range(NCORES)), trace=trace)
    outs = [res.results[i]["out"].T for i in range(NCORES)]   # [O,BL] -> [BL,O]
    return np.concatenate(outs, axis=0).astype(np.float32), res


def kernel(x, W_dense, b_dense, tau_n, tau_m, W_ro, b_ro, tau_m_ro):
    out, _ = run(dict(x=x, W_dense=W_dense, b_dense=b_dense, tau_n=tau_n,
                      tau_m=tau_m, W_ro=W_ro, b_ro=b_ro, tau_m_ro=tau_m_ro))
    return out


def make_bench(inputs, nc=None, prep_kwargs=None):
    """Build a timed runner with device-resident inputs (for test.py only).

    Mirrors bass2jax.run_bass_via_pjrt's multi-core path, but device_puts the
    inputs once so repeated calls measure device execution, not host transfer.
    """
    import jax
    import numpy as np_
    from jax.sharding import Mesh, PartitionSpec, NamedSharding
    from jax.experimental.shard_map import shard_map
    import concourse.mybir as mybir_
    from concourse import bass2jax

    in_maps = prepare_inputs(**inputs, **(prep_kwargs or {}))
    t_len = np_.asarray(inputs["x"]).shape[1]
    if nc is None:
        nc = _get_module(t_len)
    bass2jax.install_neuronx_cc_hook()

    partition_name = nc.partition_id_tensor.name if nc.partition_id_tensor else None
    in_names, out_names, out_avals, zero_outs = [], [], [], []
    for alloc in nc.m.functions[0].allocations:
        if not isinstance(alloc, mybir_.MemoryLocationSet):
            continue
        name = alloc.memorylocations[0].name
        if alloc.kind == "ExternalInput":
            if name != partition_name:
                in_names.append(name)
        elif alloc.kind == "ExternalOutput":
            shape = tuple(alloc.tensor_shape)
            dtype = mybir_.dt.np(alloc.dtype)
            out_names.append(name)
            out_avals.append(jax.core.ShapedArray(shape, dtype))
            zero_outs.append(np_.zeros(shape, dtype))
    n_params = len(in_names)
    all_in_names = in_names + out_names
    if partition_name is not None:
        all_in_names.append(partition_name)
    donate = tuple(range(n_params, n_params + len(out_names)))

    def _body(*args):
        operands = list(args)
        if partition_name is not None:
            operands.append(bass2jax.partition_id_tensor())
        outs = bass2jax._bass_exec_p.bind(
            *operands,
            out_avals=tuple(out_avals),
            in_names=tuple(all_in_names),
            out_names=tuple(out_names),
            lowering_input_output_aliases=(),
            sim_require_finite=True,
            sim_require_nnan=True,
            nc=nc,
        )
        return tuple(outs)

    devices = jax.devices()[:NCORES]
    mesh = Mesh(np_.asarray(devices), ("core",))
    in_specs = (PartitionSpec("core"),) * (n_params + len(out_names))
    out_specs = (PartitionSpec("core"),) * len(out_names)
    sharded = jax.jit(
        shard_map(_body, mesh=mesh, in_specs=in_specs, out_specs=out_specs,
                  check_rep=False),
        donate_argnums=donate, keep_unused=True,
    )
    concat_in = [
        np_.concatenate([np_.asarray(in_maps[c][name]) for c in range(NCORES)], axis=0)
        for name in in_names
    ]
    sh = NamedSharding(mesh, PartitionSpec("core"))
    dev_in = [jax.device_put(a, sh) for a in concat_in]

    def call():
        zeros = [np_.zeros((NCORES * z.shape[0], *z.shape[1:]), z.dtype)
                 for z in zero_outs]
        outs = sharded(*dev_in, *zeros)
        jax.block_until_ready(outs)
        return outs

    return call

